# revision 29
# baseline (speedup 1.0000x reference)
"""Trainium2 kernel for nn_Direction: out = input @ Q.T, Q from QR(weight + 1e-8).

Strategy (LAYOUT="inv2", the measured-best structure):
  - Host: QR of the small 512x512 weight (jax-on-CPU fp32), cast input + Q.T
    to fp16 (rel err 3.59e-4 vs the 2e-2 gate), pack per core into ONE flat
    DRAM tensor: Q's n=0 slice + the first batch tile first (so the first
    ring push carries only the 0.625MB the first matmul group needs), then
    the remaining Q slices, then per-iteration A.T blocks following a
    batch-tile ladder BTS = (512,1024,2048x6,1024,1024,512).
  - Device (8 cores, data-parallel over batch): Q-stationary inverted
    matmul; 512 matmuls of 512 moving batch-columns each stream at the warm
    PE floor (216 ns/MM = 512/2.4GHz + NX overhead), measured via ntff
    trace. Head: 12 dependency-free prewarm matmuls (on a gpsimd-memset
    dummy tile) keep the PE busy from the end of its preamble (~6.5us) so
    the HAM clock un-throttles (1.2->2.4 GHz) right as real data lands
    (~11us). Tail: per-n-tile stores + the tapered ladder keep the final
    store at 128KB. PSUM: 4x 2-bank tiles; evictions cast fp32->fp16 on
    the vector engine (vector-only measured ~0.7us faster than alternating
    vector/scalar and drops the ACT_TABLE_LOAD). All loads+stores ride the
    sync HWDGE ring (it fans across all 16 DMA engines at ~24GB/s each;
    scalar/gpsimd rings measured far slower). ain_bufs=6 input prefetch
    cushions the intermittent slow-DMA regime. Host unpacks to fp32.

Measured (ntff exec_time on core 0), chip cool: ~128-130 us = ~6.5us fixed
NEFF preamble + ~4.5us first-data latency + ~111us matmul stream (the
hard 16-bit floor: 512 MMs x 216 ns; fp8 DoubleRow would halve this but
e4m3 quantization measures 3.8e-2 rel err, over the 2e-2 gate) + ~2us
store drain + ~3us fixed teardown. Under the firmware power throttle (P0,
PE ~2.0GHz, DMA also ~2x slower; recurs intermittently under sustained
load) the same kernel measures ~134-154us. Prior-session baseline with
uniform 2048 tiles, separate Q load, and per-iteration 2MB stores:
157845 ns graded (head ~16us: vector-memset prewarm blocked on the DVE
preamble so HAM re-throttled mid-start; tail ~12us: last 2MB store fully
exposed).
"""

import numpy as np

import concourse.bacc as bacc
import concourse.mybir as mybir
import concourse.tile as tile
from concourse.bass_utils import run_bass_kernel_spmd

B_FULL = 131072
D = 512
N_CORES = 8
B_LOC = B_FULL // N_CORES  # 16384
P = 128
BT = 512  # batch rows per loop iteration
KT = D // P  # 4 k-tiles
SB = BT // P  # 4 psum sub-tiles per iteration

# precision mode: "fp32" | "fp16" | "bf16" | "fp16x2"
# fp16 and bf16 are both 2 bytes and measure identical (the PE streams 1
# col/cycle for either); fp16 keeps the larger precision margin
# (rel err 3.59e-4 vs bf16's 2.36e-3, gate 2e-2).
MODE = "fp16"
# kernel structure: "plain" | "packed" | "inv" | "inv2"
LAYOUT = "inv2"
BT_TILE = 2048  # batch rows per loop iteration (packed/inv layouts)
BUILD_KW = dict(pchunk=2, ps_bufs=4, ain_bufs=4, aout_bufs=4, evict="alt",
                prewarm=8)
# inv2: per-iteration batch-tile ladder (sum must be B_LOC) + build options.
# Small first tile -> first real matmul ~3us earlier; tapered tail -> the
# final stores drain behind the last matmuls instead of after them.
BTS = (512, 1024) + (2048,) * 6 + (1024, 1024, 512)
# ain_bufs=6: extra input-prefetch runway; neutral when the chip is cool,
# and cushions the intermittent slow-DMA (P0) regime where 2MB tiles only
# just keep up with the matmul stream (432ns input-wait bubbles at
# iteration boundaries, and one observed 3.5us stall that re-throttled HAM).
BUILD_KW2 = dict(pchunk=2, ps_bufs=4, ain_bufs=6, aout_bufs=8, evict="vector",
                 prewarm=12, store="n")

_DT = {
    "fp32": mybir.dt.float32,
    "fp16": mybir.dt.float16,
    "bf16": mybir.dt.bfloat16,
    "fp16x2": mybir.dt.float16,
}

# (a_input, q_input) matmul passes, accumulated in PSUM.
_PASSES = {
    "fp32": [("a0", "q0")],
    "fp16": [("a0", "q0")],
    "bf16": [("a0", "q0")],
    "fp16x2": [("a0", "q0"), ("a1", "q0"), ("a0", "q1")],
}

_CACHE = {}


def _build(mode, b_loc, reps=1, dynamic=False, bt=BT, ain_bufs=4, aout_bufs=4,
           ps_bufs=8, evict="alt", out16=True, do_in=True, do_out=True,
           out_q="sync", groups_mult=1, no_evict=False, bench_internal=False,
           samew=False):
    dt_in = _DT[mode]
    dt_out = mybir.dt.float16 if out16 else mybir.dt.float32
    passes = _PASSES[mode]
    a_names = sorted({a for a, _ in passes})
    q_names = sorted({q for _, q in passes})
    n_iter = b_loc // bt
    sb_n = bt // P

    nc = bacc.Bacc("TRN2", target_bir_lowering=False, debug=False)
    kin = "Internal" if bench_internal else "ExternalInput"
    kout = "Internal" if bench_internal else "ExternalOutput"
    a_dram = {
        n: nc.dram_tensor(n, [D, b_loc], dt_in, kind=kin).ap()
        for n in a_names
    }
    q_dram = {
        n: nc.dram_tensor(n, [D, D], dt_in, kind=kin).ap()
        for n in q_names
    }
    out_dram = nc.dram_tensor("out", [b_loc, D], dt_out, kind=kout).ap()
    if bench_internal:
        seed_dram = nc.dram_tensor(
            "seed", [1, 64], mybir.dt.int32, kind="ExternalInput"
        ).ap()
        dout_dram = nc.dram_tensor(
            "dout", [1, 64], mybir.dt.int32, kind="ExternalOutput"
        ).ap()

    with tile.TileContext(nc) as tc:
        with (
            tc.tile_pool(name="consts", bufs=1) as consts,
            tc.tile_pool(name="ain", bufs=ain_bufs) as ain,
            tc.tile_pool(name="aout", bufs=aout_bufs) as aout,
            tc.tile_pool(name="ps", bufs=ps_bufs, space="PSUM") as ps_pool,
        ):
            q_tiles = {}
            for qn in q_names:
                qt = consts.tile([P, KT, D], dt_in, name=f"qt_{qn}")
                nc.sync.dma_start(
                    out=qt[:, :, :],
                    in_=q_dram[qn].rearrange("(k p) n -> p k n", p=P),
                )
                q_tiles[qn] = qt

            a_const = {}
            if not do_in:
                for an in a_names:
                    at = consts.tile([P, KT, bt], dt_in, name=f"ac_{an}")
                    src = a_dram[an].rearrange("(k p) b -> p k b", p=P)[:, :, 0:bt]
                    nc.sync.dma_start(out=at[:, :, :], in_=src)
                    a_const[an] = at

            out_eng = {"sync": nc.sync, "scalar": nc.scalar}[out_q]

            def body():
                for it in range(n_iter):
                    a_tiles = {}
                    for an in a_names:
                        if not do_in:
                            a_tiles[an] = a_const[an]
                            continue
                        at = ain.tile(
                            [P, KT, bt], dt_in, name=f"at_{an}", tag=f"at_{an}"
                        )
                        src = a_dram[an].rearrange("(k p) b -> p k b", p=P)[
                            :, :, it * bt : (it + 1) * bt
                        ]
                        nc.sync.dma_start(out=at[:, :, :], in_=src)
                        a_tiles[an] = at
                    for sb in range(sb_n):
                        ps = ps_pool.tile(
                            [P, D], mybir.dt.float32, name="ps", tag="ps"
                        )
                        n_mm = len(passes) * KT * groups_mult
                        mm = 0
                        for _g in range(groups_mult):
                            for an, qn in passes:
                                at = a_tiles[an]
                                qt = q_tiles[qn]
                                for k in range(KT):
                                    stat = (
                                        at[:, 0, 0:P]
                                        if samew
                                        else at[:, k, sb * P : (sb + 1) * P]
                                    )
                                    nc.tensor.matmul(
                                        ps[:, :],
                                        stat,
                                        qt[:, k, :],
                                        start=(mm == 0),
                                        stop=(mm == n_mm - 1),
                                    )
                                    mm += 1
                        if no_evict:
                            continue
                        ot = aout.tile([P, D], dt_out, name="ot", tag="ot")
                        if evict == "any":
                            nc.any.tensor_copy(ot[:, :], ps[:, :])
                        elif evict == "vector":
                            nc.vector.tensor_copy(ot[:, :], ps[:, :])
                        elif evict == "alt":
                            if sb % 2 == 0:
                                nc.vector.tensor_copy(ot[:, :], ps[:, :])
                            else:
                                nc.scalar.activation(
                                    ot[:, :],
                                    ps[:, :],
                                    mybir.ActivationFunctionType.Copy,
                                )
                        b0 = it * bt + sb * P
                        if do_out:
                            out_eng.dma_start(
                                out=out_dram[b0 : b0 + P, :], in_=ot[:, :]
                            )

            if bench_internal:
                st = consts.tile([1, 64], mybir.dt.int32, name="seed_t")
                nc.sync.dma_start(out=st[:, :], in_=seed_dram[:, :])
                nc.sync.dma_start(out=dout_dram[:, :], in_=st[:, :])

            if dynamic == "unroll" and reps > 1:
                tc.For_i_unrolled(0, reps, 1, lambda iv: body(), max_unroll=4)
            elif dynamic and reps > 1:
                with tc.For_i(0, reps, 1):
                    body()
            else:
                for _ in range(reps):
                    body()

    nc.compile()
    return nc


def _build_packed(mode, b_loc, reps=1, dynamic=False, bt=2048, ain_bufs=3,
                  aout_bufs=3, ps_bufs=8, evict="alt", out_q="scalar",
                  bench_internal=False, pb=1, do_in=True, do_out=True,
                  no_evict=False):
    """Single-pass matmul with host-packed DRAM layouts.

    a_dram[p, it, k, b] = A.T[k*128+p, it*bt+b]  (fp16/bf16) — each per-iter
    input DMA reads KT*bt*2 bytes fully contiguous per partition.
    out_dram[p, it, s, n] = out[it*bt + s*128 + p, n] (fp16) — each per-iter
    output DMA writes sb_n*D*2 bytes fully contiguous per partition; the host
    unpermutes. One dma_start each way per iteration; outputs go on the
    scalar HWDGE ring so stores never head-of-line-block input loads on the
    sync ring.
    """
    dt_in = _DT[mode]
    dt_out = mybir.dt.float16
    assert len(_PASSES[mode]) == 1, "packed build supports single-pass modes"
    n_iter = b_loc // bt
    sb_n = bt // P

    nc = bacc.Bacc("TRN2", target_bir_lowering=False, debug=False)
    kin = "Internal" if bench_internal else "ExternalInput"
    kout = "Internal" if bench_internal else "ExternalOutput"
    a_dram = nc.dram_tensor("a0", [P, n_iter, KT, bt], dt_in, kind=kin).ap()
    q_dram = nc.dram_tensor("q0", [D, D], dt_in, kind=kin).ap()
    out_dram = nc.dram_tensor(
        "out", [P, n_iter, sb_n, D], dt_out, kind=kout
    ).ap()
    if bench_internal:
        seed_dram = nc.dram_tensor(
            "seed", [1, 64], mybir.dt.int32, kind="ExternalInput"
        ).ap()
        dout_dram = nc.dram_tensor(
            "dout", [1, 64], mybir.dt.int32, kind="ExternalOutput"
        ).ap()

    with tile.TileContext(nc) as tc:
        with (
            tc.tile_pool(name="consts", bufs=1) as consts,
            tc.tile_pool(name="ain", bufs=ain_bufs) as ain,
            tc.tile_pool(name="aout", bufs=aout_bufs) as aout,
            tc.tile_pool(name="ps", bufs=ps_bufs, space="PSUM") as ps_pool,
        ):
            qt = consts.tile([P, KT, D], dt_in, name="qt")
            nc.sync.dma_start(
                out=qt[:, :, :],
                in_=q_dram.rearrange("(k p) n -> p k n", p=P),
            )
            out_eng = {"sync": nc.sync, "scalar": nc.scalar}[out_q]

            a_res = None
            if not do_in:
                a_res = consts.tile([P, KT, bt], dt_in, name="a_res")
                nc.sync.dma_start(out=a_res[:, :, :], in_=a_dram[:, 0, :, :])

            assert sb_n % pb == 0 and pb * ps_bufs <= 8

            def body():
                for it in range(n_iter):
                    if do_in:
                        at = ain.tile([P, KT, bt], dt_in, name="at", tag="at")
                        nc.sync.dma_start(
                            out=at[:, :, :], in_=a_dram[:, it, :, :]
                        )
                    else:
                        at = a_res
                    ot = aout.tile([P, sb_n, D], dt_out, name="ot", tag="ot")
                    for g in range(sb_n // pb):
                        ps = ps_pool.tile(
                            [P, pb, D], mybir.dt.float32, name="ps", tag="ps"
                        )
                        for j in range(pb):
                            sb = g * pb + j
                            for k in range(KT):
                                nc.tensor.matmul(
                                    ps[:, j, :],
                                    at[:, k, sb * P : (sb + 1) * P],
                                    qt[:, k, :],
                                    start=(k == 0),
                                    stop=(k == KT - 1),
                                )
                        if no_evict:
                            continue
                        dst = ot[:, g * pb : (g + 1) * pb, :]
                        if evict == "alt" and g % 2 == 1:
                            nc.scalar.activation(
                                dst,
                                ps[:, :, :],
                                mybir.ActivationFunctionType.Copy,
                            )
                        elif evict == "scalar":
                            nc.scalar.activation(
                                dst,
                                ps[:, :, :],
                                mybir.ActivationFunctionType.Copy,
                            )
                        else:
                            nc.vector.tensor_copy(dst, ps[:, :, :])
                    if do_out and not no_evict:
                        out_eng.dma_start(
                            out=out_dram[:, it, :, :], in_=ot[:, :, :]
                        )

            if bench_internal:
                st = consts.tile([1, 64], mybir.dt.int32, name="seed_t")
                nc.sync.dma_start(out=st[:, :], in_=seed_dram[:, :])
                nc.sync.dma_start(out=dout_dram[:, :], in_=st[:, :])

            if dynamic and reps > 1:
                with tc.For_i(0, reps, 1):
                    body()
            else:
                for _ in range(reps):
                    body()

    nc.compile()
    return nc


def _build_inv(mode, b_loc, reps=1, dynamic=False, bt=2048, ain_bufs=3,
               aout_bufs=3, ps_bufs=2, evict="alt", out_q="sync",
               bench_internal=False, expldw=False, do_in=True, do_out=True,
               no_evict=False, pchunk=None, prewarm=0):
    """Q-stationary inverted matmul: out.T tiles in PSUM.

    For each 128-wide n-tile of Q, the stationary operand qt[:, k, n-slice]
    is reused by `bt/512` consecutive matmuls streaming A chunks (moving,
    N=512 batch cols), amortizing the PE weight load. PSUM tile = [128(n),
    chunks, 512(b)] fp32 spanning `chunks` banks; 16-MM accumulation groups.
    Output lands transposed; host unpacks.

    a_dram[p, it, k, b] = A.T[k*128+p, it*bt+b] (same as packed layout).
    out_dram[p, n, it, b] = out[it*bt + b, n*128 + p].
    """
    dt_in = _DT[mode]
    dt_out = mybir.dt.float16
    assert len(_PASSES[mode]) == 1
    n_iter = b_loc // bt
    ch = bt // 512  # moving chunks per iteration
    nt = D // P  # 4 n-tiles
    pchunk = pchunk or ch  # chunks per PSUM tile (eviction granularity)
    assert ch % pchunk == 0
    assert pchunk * ps_bufs <= 8

    nc = bacc.Bacc("TRN2", target_bir_lowering=False, debug=False)
    kin = "Internal" if bench_internal else "ExternalInput"
    kout = "Internal" if bench_internal else "ExternalOutput"
    a_dram = nc.dram_tensor("a0", [P, n_iter, KT, bt], dt_in, kind=kin).ap()
    q_dram = nc.dram_tensor("q0", [D, D], dt_in, kind=kin).ap()
    out_dram = nc.dram_tensor(
        "out", [P, nt, n_iter, bt], dt_out, kind=kout
    ).ap()
    if bench_internal:
        seed_dram = nc.dram_tensor(
            "seed", [1, 64], mybir.dt.int32, kind="ExternalInput"
        ).ap()
        dout_dram = nc.dram_tensor(
            "dout", [1, 64], mybir.dt.int32, kind="ExternalOutput"
        ).ap()

    with tile.TileContext(nc) as tc:
        with (
            tc.tile_pool(name="consts", bufs=1) as consts,
            tc.tile_pool(name="ain", bufs=ain_bufs) as ain,
            tc.tile_pool(name="aout", bufs=aout_bufs) as aout,
            tc.tile_pool(name="ps", bufs=ps_bufs, space="PSUM") as ps_pool,
        ):
            qt = consts.tile([P, KT, D], dt_in, name="qt")
            nc.sync.dma_start(
                out=qt[:, :, :],
                in_=q_dram.rearrange("(k p) n -> p k n", p=P),
            )
            out_eng = {
                "sync": nc.sync, "scalar": nc.scalar, "gpsimd": nc.gpsimd,
            }[out_q]

            if prewarm:
                # Dummy matmuls on an unwritten SBUF tile: no input deps, so
                # they issue immediately and warm the PE clock (HAM) out of
                # its 1.2 GHz idle state while the first A tile streams in.
                # Output group is never read; the pool slot is recycled.
                wsrc = consts.tile([P, 512], dt_in, name="warm_src")
                nc.vector.memset(wsrc[:, :], 0)
                wps = ps_pool.tile([P, pchunk, 512], mybir.dt.float32,
                                   name="ps", tag="ps")
                for i in range(prewarm):
                    nc.tensor.matmul(
                        wps[:, i % pchunk, :],
                        wsrc[:, 0:P],
                        wsrc[:, :],
                        start=(i < pchunk),
                        stop=(i >= prewarm - pchunk),
                    )

            a_res = None
            if not do_in:
                a_res = consts.tile([P, KT, bt], dt_in, name="a_res")
                nc.sync.dma_start(out=a_res[:, :, :], in_=a_dram[:, 0, :, :])

            def body():
                for it in range(n_iter):
                    if do_in:
                        at = ain.tile([P, KT, bt], dt_in, name="at", tag="at")
                        nc.sync.dma_start(
                            out=at[:, :, :], in_=a_dram[:, it, :, :]
                        )
                    else:
                        at = a_res
                    ot = aout.tile([P, nt, bt], dt_out, name="ot", tag="ot")
                    n_ps = ch // pchunk
                    for n in range(nt):
                        pss = [
                            ps_pool.tile(
                                [P, pchunk, 512], mybir.dt.float32,
                                name="ps", tag="ps",
                            )
                            for _ in range(n_ps)
                        ]
                        for k in range(KT):
                            w = qt[:, k, n * P : (n + 1) * P]
                            if expldw:
                                nc.tensor.ldweights(w)
                            for c in range(ch):
                                mm = nc.tensor.matmul(
                                    pss[c // pchunk][:, c % pchunk, :],
                                    w,
                                    at[:, k, c * 512 : (c + 1) * 512],
                                    start=(k == 0),
                                    stop=(k == KT - 1),
                                )
                                if expldw:
                                    mm.ins.ldweights = False
                        if no_evict:
                            continue
                        for t in range(n_ps):
                            dst = ot[:, n, t * pchunk * 512 :
                                     (t + 1) * pchunk * 512]
                            if evict == "alt" and (n * n_ps + t) % 2 == 1:
                                nc.scalar.activation(
                                    dst,
                                    pss[t][:, :, :],
                                    mybir.ActivationFunctionType.Copy,
                                )
                            else:
                                nc.vector.tensor_copy(dst, pss[t][:, :, :])
                    if do_out and not no_evict:
                        out_eng.dma_start(
                            out=out_dram[:, :, it, :], in_=ot[:, :, :]
                        )

            if bench_internal:
                st = consts.tile([1, 64], mybir.dt.int32, name="seed_t")
                nc.sync.dma_start(out=st[:, :], in_=seed_dram[:, :])
                nc.sync.dma_start(out=dout_dram[:, :], in_=st[:, :])

            if dynamic and reps > 1:
                with tc.For_i(0, reps, 1):
                    body()
            else:
                for _ in range(reps):
                    body()

    nc.compile()
    return nc


def _build_inv2(mode, b_loc, reps=1, dynamic=False, bts=None, ain_bufs=4,
                aout_bufs=4, ps_bufs=4, pchunk=2, evict="alt", prewarm=10,
                store="iter", q_ring="sync", in_ring="sync", out_ring="sync",
                bench_internal=False, do_in=True, do_out=True,
                no_evict=False):
    """Q-stationary inverted matmul, v2: flat packed DRAM layouts with a
    per-iteration batch-tile ladder.

    - bts: tuple of per-iteration batch sizes (multiples of 512, sum=b_loc).
      Small first tile -> the first real matmul starts ~4us earlier; small
      last tile -> the final store drains in ~1us instead of ~8.
    - a_dram[p, koff_it + k*bt + b] = A[off_it + b, k*128 + p]: each
      iteration's input DMA is KT*bt*2B fully contiguous per partition.
    - q_dram[p, k*512 + m] = Q[m, k*128 + p]: 4KB contiguous per partition
      (the old (k p) n layout produced 1KB packets and a ~4us Q load).
    - out_dram[p, noff_it + n*bt + b] = out[off_it + b, n*128 + p]; stores
      are pushed per iteration (store="iter") or per n-tile (store="n").
    - prewarm: dummy matmuls on an *unwritten* SBUF tile (no memset: the
      baseline's vector memset waited on the DVE preamble until ~6.8us, so
      the PE sat idle and HAM re-throttled; without deps the warmup issues
      right after the PE's preamble branch at ~6.5us).
    """
    dt_in = _DT[mode]
    dt_out = mybir.dt.float16
    assert len(_PASSES[mode]) == 1
    bts = list(bts or (b_loc // 2048) * (2048,))
    assert sum(bts) == b_loc and all(bt % 512 == 0 for bt in bts)
    nt = D // P  # 4 n-tiles

    nc = bacc.Bacc("TRN2", target_bir_lowering=False, debug=False)
    kin = "Internal" if bench_internal else "ExternalInput"
    kout = "Internal" if bench_internal else "ExternalOutput"
    # a0 carries Q (first KT*D elements per partition) + the A blocks, so Q
    # and the first batch tile arrive in ONE DMA (one ring push, one
    # contiguous read per partition) instead of two serialized ones.
    a_dram = nc.dram_tensor(
        "a0", [P, KT * D + KT * b_loc], dt_in, kind=kin
    ).ap()
    out_dram = nc.dram_tensor("out", [P, nt * b_loc], dt_out, kind=kout).ap()
    if bench_internal:
        seed_dram = nc.dram_tensor(
            "seed", [1, 64], mybir.dt.int32, kind="ExternalInput"
        ).ap()
        dout_dram = nc.dram_tensor(
            "dout", [1, 64], mybir.dt.int32, kind="ExternalOutput"
        ).ap()

    with tile.TileContext(nc) as tc:
        with (
            tc.tile_pool(name="consts", bufs=1) as consts,
            tc.tile_pool(name="ain", bufs=ain_bufs) as ain,
            tc.tile_pool(name="aout", bufs=aout_bufs) as aout,
            tc.tile_pool(name="ps", bufs=ps_bufs, space="PSUM") as ps_pool,
        ):
            rings = {"sync": nc.sync, "scalar": nc.scalar, "vector": nc.vector,
                     "gpsimd": nc.gpsimd}
            q_eng = rings[q_ring]
            out_eng = rings[out_ring]

            # Split-Q head: the first ring push carries only Q's n=0 slice
            # (KT*128 cols) + the first batch tile -- 0.625MB instead of
            # 1MB -- so the first matmul group starts ~1.5us earlier; the
            # remaining Q slices (n=1..3) follow in a second push and land
            # while group n=0 computes.
            bt0 = bts[0]
            qat = consts.tile([P, KT * P + KT * bt0], dt_in, name="qat")
            q_eng.dma_start(
                out=qat[:, :], in_=a_dram[:, 0 : KT * P + KT * bt0]
            )
            at0 = qat[:, KT * P : KT * P + KT * bt0]
            q_rest = consts.tile([P, (nt - 1) * KT * P], dt_in, name="q_rest")
            q_eng.dma_start(
                out=q_rest[:, :],
                in_=a_dram[:, KT * P + KT * bt0 : KT * D + KT * bt0],
            )

            def wslice(n, k):
                # stationary operand for (n, k): 128 Q columns, packed
                # n-major so each slice is contiguous per partition
                if n == 0:
                    return qat[:, k * P : (k + 1) * P]
                base = (n - 1) * KT * P
                return q_rest[:, base + k * P : base + (k + 1) * P]

            if prewarm:
                # gpsimd memset: its preamble ends earliest (~5.9us), so the
                # prewarm matmuls can issue right as the PE leaves its own
                # preamble (~6.5us); a vector memset waits until ~6.8us.
                wsrc = consts.tile([P, 512], dt_in, name="warm_src")
                nc.gpsimd.memset(wsrc[:, :], 0)
                wps = ps_pool.tile([P, pchunk, 512], mybir.dt.float32,
                                   name="ps", tag="ps")
                for i in range(prewarm):
                    nc.tensor.matmul(
                        wps[:, i % pchunk, :],
                        wsrc[:, 0:P],
                        wsrc[:, :],
                        start=(i < pchunk),
                        stop=(i >= prewarm - pchunk),
                    )

            def body():
                koff = KT * (D + bt0)
                noff = 0
                ev = 0
                for it, bt in enumerate(bts):
                    ch = bt // 512
                    pc = min(pchunk, ch)
                    if it == 0 or not do_in:
                        at = at0
                        bt = bt0
                        ch = bt // 512
                        pc = min(pchunk, ch)
                    else:
                        at = ain.tile([P, KT * bt], dt_in, name="at", tag="at")
                        in_eng = rings[in_ring] if in_ring != "alt" else (
                            nc.sync if it % 2 == 0 else nc.gpsimd)
                        in_eng.dma_start(
                            out=at[:, :],
                            in_=a_dram[:, koff : koff + KT * bt],
                        )
                    if store != "n":
                        ot = aout.tile([P, nt * bt], dt_out, name="ot",
                                       tag="ot")
                    for n in range(nt):
                        if store == "n":
                            ot = aout.tile([P, bt], dt_out, name="ot",
                                           tag="ot")
                            obase = 0
                        else:
                            obase = n * bt
                        n_ps = ch // pc
                        pss = [
                            ps_pool.tile([P, pc, 512], mybir.dt.float32,
                                         name="ps", tag="ps")
                            for _ in range(n_ps)
                        ]
                        for k in range(KT):
                            w = wslice(n, k)
                            for c in range(ch):
                                nc.tensor.matmul(
                                    pss[c // pc][:, c % pc, :],
                                    w,
                                    at[:, k * bt + c * 512 :
                                       k * bt + (c + 1) * 512],
                                    start=(k == 0),
                                    stop=(k == KT - 1),
                                )
                        if no_evict:
                            continue
                        for t in range(n_ps):
                            dst = ot[:, obase + t * pc * 512 :
                                     obase + (t + 1) * pc * 512]
                            if evict == "alt" and ev % 2 == 1:
                                nc.scalar.activation(
                                    dst,
                                    pss[t][:, :, :],
                                    mybir.ActivationFunctionType.Copy,
                                )
                            else:
                                nc.vector.tensor_copy(dst, pss[t][:, :, :])
                            ev += 1
                        if store == "n" and do_out:
                            out_eng.dma_start(
                                out=out_dram[:, noff + n * bt :
                                             noff + (n + 1) * bt],
                                in_=ot[:, :],
                            )
                    if store == "iter" and do_out and not no_evict:
                        out_eng.dma_start(
                            out=out_dram[:, noff : noff + nt * bt],
                            in_=ot[:, :],
                        )
                    if it > 0:
                        koff += KT * bt
                    noff += nt * bt

            if bench_internal:
                st = consts.tile([1, 64], mybir.dt.int32, name="seed_t")
                nc.sync.dma_start(out=st[:, :], in_=seed_dram[:, :])
                nc.sync.dma_start(out=dout_dram[:, :], in_=st[:, :])

            if dynamic and reps > 1:
                with tc.For_i(0, reps, 1):
                    body()
            else:
                for _ in range(reps):
                    body()

    nc.compile()
    return nc


def _prep_inputs_inv2(mode, input_np, qt_np, n_cores, b_loc, bts):
    """Flat packed per-core input maps for the inv2 layout.

    a0[p, :KT*D] = packed Q; a0[p, KT*D + koff + k*bt + b] = A[off+b, k*128+p]
    """
    cast_dt = {"fp32": np.float32, "fp16": np.float16}[mode] \
        if mode != "bf16" else None
    if cast_dt is None:
        import ml_dtypes

        cast_dt = ml_dtypes.bfloat16
    nt = D // P
    # n-major packed Q: q_all[p, n*KT*128 + k*128 + j] = Q[n*128+j, k*128+p]
    q_all = np.ascontiguousarray(
        qt_np.reshape(KT, P, nt, P).transpose(1, 2, 0, 3).reshape(P, KT * D)
    ).astype(cast_dt)
    q_n0 = q_all[:, : KT * P]
    q_rest = q_all[:, KT * P :]
    inp_c = input_np.astype(cast_dt)
    maps = []
    for i in range(n_cores):
        a = inp_c[i * b_loc : (i + 1) * b_loc]
        blocks = []
        off = 0
        for bt in bts:
            blk = a[off : off + bt]  # [bt, 512]
            # [bt, KT, P] -> [P, KT, bt] -> [P, KT*bt]
            blocks.append(
                blk.reshape(bt, KT, P).transpose(2, 1, 0).reshape(P, KT * bt)
            )
            off += bt
        packed = np.ascontiguousarray(np.concatenate(
            [q_n0, blocks[0], q_rest] + blocks[1:], axis=1))
        maps.append({"a0": packed})
    return maps


def _unpack_out_inv2(res, n_cores, b_loc, bts):
    """out_dram[p, noff + n*bt + b] -> (n_cores*b_loc, D) fp32."""
    nt = D // P
    outs = []
    for i in range(n_cores):
        o = np.asarray(res.results[i]["out"])  # [P, nt*b_loc]
        rows = np.empty((b_loc, D), dtype=np.float16)
        off = 0
        noff = 0
        for bt in bts:
            blk = o[:, noff : noff + nt * bt].reshape(P, nt, bt)
            # out[off+b, n*128+p] = blk[p, n, b]
            rows[off : off + bt] = blk.transpose(2, 1, 0).reshape(bt, D)
            off += bt
            noff += nt * bt
        outs.append(rows)
    return np.concatenate(outs, axis=0).astype(np.float32)


def _unpack_out_inv(res, n_cores, b_loc, bt=2048):
    """out_dram[p, n, it, b] -> (n_cores*b_loc, D) fp32."""
    n_iter = b_loc // bt
    outs = []
    for i in range(n_cores):
        o = np.asarray(res.results[i]["out"])  # [P, nt, n_iter, bt]
        # out[it*bt + b, n*128 + p] = o[p, n, it, b]
        o = o.transpose(2, 3, 1, 0).reshape(b_loc, D)
        outs.append(o)
    return np.concatenate(outs, axis=0).astype(np.float32)


def _get_nc(mode, b_loc, **kw):
    return _get_nc_reps(mode, b_loc, 1, **kw)


def _get_nc_reps(mode, b_loc, reps, dynamic=False, layout="plain", **kw):
    key = (mode, b_loc, reps, dynamic, layout, tuple(sorted(kw.items())))
    if key not in _CACHE:
        builder = {
            "plain": _build,
            "packed": _build_packed,
            "inv": _build_inv,
            "inv2": _build_inv2,
        }[layout]
        _CACHE[key] = builder(mode, b_loc, reps, dynamic, **kw)
    return _CACHE[key]


def _split16(x):
    hi = x.astype(np.float16)
    lo = (x - hi.astype(np.float32)).astype(np.float16)
    return hi, lo


def _prep_inputs(mode, input_np, qt_np, n_cores, b_loc):
    """Build per-core input maps. input_np: (n_cores*b_loc, D) fp32 row-major.
    qt_np: (D, D) fp32, qt_np[m, n] = Q[n, m]."""
    maps = []
    if mode == "fp16x2":
        qh, ql = _split16(qt_np)
        for i in range(n_cores):
            at = np.ascontiguousarray(input_np[i * b_loc : (i + 1) * b_loc].T)
            ah, al = _split16(at)
            maps.append({"a0": ah, "a1": al, "q0": qh, "q1": ql})
    else:
        if mode == "bf16":
            import ml_dtypes

            cast_dt = ml_dtypes.bfloat16
        else:
            cast_dt = {"fp32": np.float32, "fp16": np.float16}[mode]
        q0 = qt_np.astype(cast_dt)
        # cast first (vectorized over the full row-major array), then
        # transpose per-core shards
        inp_c = input_np.astype(cast_dt)
        for i in range(n_cores):
            at = np.ascontiguousarray(inp_c[i * b_loc : (i + 1) * b_loc].T)
            maps.append({"a0": at, "q0": q0})
    return maps


def _prep_inputs_packed(mode, input_np, qt_np, n_cores, b_loc, bt=2048):
    """Packed per-core input maps: a0[p, it, k, b] = A.T[k*128+p, it*bt+b]."""
    if mode == "bf16":
        import ml_dtypes

        cast_dt = ml_dtypes.bfloat16
    else:
        cast_dt = {"fp32": np.float32, "fp16": np.float16}[mode]
    n_iter = b_loc // bt
    q0 = qt_np.astype(cast_dt)
    inp_c = input_np.astype(cast_dt)
    maps = []
    for i in range(n_cores):
        a = inp_c[i * b_loc : (i + 1) * b_loc]
        # a[it*bt + b, k*128 + p] -> [p, it, k, b]
        packed = np.ascontiguousarray(
            a.reshape(n_iter, bt, KT, P).transpose(3, 0, 2, 1)
        )
        maps.append({"a0": packed, "q0": q0})
    return maps


def _unpack_out(res, n_cores, b_loc, bt=2048):
    """out_dram[p, it, s, n] -> (n_cores*b_loc, D) fp32."""
    outs = []
    for i in range(n_cores):
        o = np.asarray(res.results[i]["out"])  # [P, n_iter, sb_n, D]
        o = o.transpose(1, 2, 0, 3).reshape(b_loc, D)
        outs.append(o)
    return np.concatenate(outs, axis=0).astype(np.float32)


def _compute_qt(weight_np):
    """Q from QR(weight + 1e-8), transposed. Prefer jax-on-CPU so Q matches the
    fp32 jax reference bit-for-bit when possible; fall back to LAPACK (both are
    Householder QR and agree to ~1e-6, so either is well within tolerance)."""
    w = weight_np.astype(np.float32)
    try:
        import jax
        import jax.numpy as jnp

        cpu = jax.devices("cpu")[0]
        with jax.default_device(cpu):
            q, _ = jnp.linalg.qr(jax.device_put(w, cpu) + 1e-8)
        q = np.asarray(q)
    except Exception:
        q, _ = np.linalg.qr(w + np.float32(1e-8))
    return np.ascontiguousarray(q.T.astype(np.float32))


def run(input_np, weight_np, mode=None, n_cores=N_CORES, b_loc=None,
        layout=None, bt=None, build_kw=None, **run_kwargs):
    mode = mode or MODE
    layout = layout or LAYOUT
    bt = bt or BT_TILE
    if build_kw is None:
        build_kw = BUILD_KW
    if mode == "fp16x2":
        layout = "plain"
    b_loc = b_loc or (input_np.shape[0] // n_cores)
    assert input_np.shape[0] == n_cores * b_loc, (
        f"batch {input_np.shape[0]} not divisible into {n_cores} cores"
    )
    assert input_np.shape[1] == D

    qt = _compute_qt(weight_np)

    if layout == "inv2":
        bts = run_kwargs.pop("bts", None) or BTS
        kw = dict(BUILD_KW2 if build_kw is BUILD_KW else build_kw)
        nc = _get_nc(mode, b_loc, layout=layout, bts=tuple(bts), **kw)
        in_maps = _prep_inputs_inv2(
            mode, np.asarray(input_np), qt, n_cores, b_loc, bts
        )
        res = run_bass_kernel_spmd(
            nc, in_maps, list(range(n_cores)), **run_kwargs
        )
        return _unpack_out_inv2(res, n_cores, b_loc, bts), res

    if layout in ("packed", "inv"):
        nc = _get_nc(mode, b_loc, layout=layout, bt=bt, **(build_kw or {}))
        in_maps = _prep_inputs_packed(
            mode, np.asarray(input_np), qt, n_cores, b_loc, bt=bt
        )
        res = run_bass_kernel_spmd(
            nc, in_maps, list(range(n_cores)), **run_kwargs
        )
        unpack = _unpack_out_inv if layout == "inv" else _unpack_out
        return unpack(res, n_cores, b_loc, bt=bt), res

    nc = _get_nc(mode, b_loc, **(build_kw or {}))
    in_maps = _prep_inputs(mode, np.asarray(input_np), qt, n_cores, b_loc)
    res = run_bass_kernel_spmd(nc, in_maps, list(range(n_cores)), **run_kwargs)
    out = np.concatenate(
        [np.asarray(res.results[i]["out"], dtype=np.float32)
         for i in range(n_cores)],
        axis=0,
    )
    return out, res


def kernel(input, weight):
    out, _ = run(
        np.asarray(input, dtype=np.float32), np.asarray(weight, dtype=np.float32)
    )
    return np.ascontiguousarray(out, dtype=np.float32)

